# revision 1
# baseline (speedup 1.0000x reference)
"""Trainium2 Bass kernel for nn_Embed_38766374814290 (embedding_lookup).

Math: out[i,j,l,e] = A[m][e] + delta_s[i,j,l] * B[m][e]
  where m = (j < traj_len[i]), delta_s = where(m, mat2[traj_loc-1], 0),
  A[m] = emb_sl_w[m] + emb_tl_w[m],
  B[m] = (emb_su_w[m]-emb_sl_w[m])/SU + (emb_tu_w[m]-emb_tl_w[m])/TU.

Sharding: pure data parallel over batch N = 32 -> 4 rows per core x 8 cores.

Per-core kernel, per batch row i (128 positions):
  1. indirect-gather G[pos, l] = mat2x[idx[pos], l] in bf16 hi/lo halves
     (idx redirects invalid positions to an appended all-zero row 4096).
  2. For each 32-wide l-group: three PE transposes pack [Ghi; Glo; Ghi]
     l-slices into one [96, 128] PSUM tile (base partitions 0/32/64);
     one DVE copy evicts it to SBUF bf16 (rows 0-95 of the lhsT tile);
     a tiny DMA writes 4 constant rows [m, m, 1, 1] (rows 96-99).
  3. Four K=100 bf16 matmuls per l-group against constant block-diagonal
     rhs build out[pos, (l,e)] = G*B1 + m*dA + A0 = G*B1 + A[m] in one
     pass (three-term hi/lo products keep rel err ~1e-5).
  4. PSUM -> SBUF evictions are pure copies, split between DVE and ACT.
  5. Output rows DMA out with fully contiguous 32KB-per-partition
     descriptors (1 MiB per DMA).
"""
import os
import numpy as np
from contextlib import ExitStack

SU, TU = 10000.0, 86400.0
N, M, L, E = 32, 128, 128, 64
NLOC = 4096
NCORES = 8
ROWS = N // NCORES  # 4 batch rows per core

_CACHE = {}


def _install_profhook():
    """Optional: shim the missing antenv.axon_hooks so trace=True works."""
    import sys
    import types
    if "antenv.axon_hooks" in sys.modules:
        return True
    try:
        from trn_agent_boot.trn_boot import _ntff_profile_via_ctypes
    except Exception:
        return False
    hook = [None]
    mod = types.ModuleType("antenv.axon_hooks")
    mod.set_axon_ntff_profile_hook = lambda h: hook.__setitem__(0, h)
    mod.get_axon_ntff_profile_hook = lambda: hook[0]
    sys.modules["antenv.axon_hooks"] = mod
    try:
        mod.set_axon_ntff_profile_hook(
            _ntff_profile_via_ctypes("/opt/axon/libaxon_pjrt.so"))
    except Exception:
        return False
    return True


def _build():
    import concourse.bass as bass
    import concourse.tile as tile
    from concourse import bacc, mybir

    F32 = mybir.dt.float32
    BF16 = mybir.dt.bfloat16
    I32 = mybir.dt.int32

    nc = bacc.Bacc("TRN2", target_bir_lowering=False, debug=False,
                   enable_asserts=True, num_devices=NCORES)
    m2hi_d = nc.dram_tensor("m2hi", [NLOC + 1, L], BF16,
                            kind="ExternalInput").ap()
    m2lo_d = nc.dram_tensor("m2lo", [NLOC + 1, L], BF16,
                            kind="ExternalInput").ap()
    idx_d = nc.dram_tensor("idx", [ROWS, M], I32, kind="ExternalInput").ap()
    mrow_d = nc.dram_tensor("mrow", [ROWS, 4, 4 * M], BF16,
                            kind="ExternalInput").ap()
    rhs_d = nc.dram_tensor("rhs", [4, 100, 8 * E], BF16,
                           kind="ExternalInput").ap()
    ident_d = nc.dram_tensor("ident", [128, 128], BF16,
                             kind="ExternalInput").ap()
    out_d = nc.dram_tensor("out", [ROWS, M, L * E], F32,
                           kind="ExternalOutput").ap()

    with tile.TileContext(nc) as tc, ExitStack() as ctx:
        const = ctx.enter_context(tc.tile_pool(name="const", bufs=1))
        ipool = ctx.enter_context(tc.tile_pool(name="idxp", bufs=2))
        gpool = ctx.enter_context(tc.tile_pool(name="gath", bufs=2))
        gtpool = ctx.enter_context(tc.tile_pool(name="gt", bufs=4))
        opool = ctx.enter_context(tc.tile_pool(name="orow", bufs=3))
        pst = ctx.enter_context(tc.tile_pool(name="pst", bufs=2, space="PSUM"))
        pso = ctx.enter_context(tc.tile_pool(name="pso", bufs=6, space="PSUM"))

        ident = const.tile([128, 128], BF16)
        nc.sync.dma_start(ident[:], ident_d[:])
        # HAM warmup: ~3.5us of back-to-back matmuls at t=0 lifts the PE
        # clock gate to 8/8 before the real burst; store-paced gaps later
        # are too short for it to re-throttle. Results are never read.
        wrhs = const.tile([128, 8 * E], BF16)
        nc.vector.memset(wrhs[:], 0.0)
        wpo = pso.tile([128, 8 * E], F32, tag="po")
        for _ in range(20):
            nc.tensor.matmul(wpo[:], lhsT=ident[:], rhs=wrhs[:],
                             start=True, stop=True)
        rhs_tiles = []
        for s in range(4):
            rt = const.tile([100, 8 * E], BF16, tag=f"rhs{s}")
            nc.sync.dma_start(rt[:], rhs_d[s])
            rhs_tiles.append(rt)

        for i in range(ROWS):
            it = ipool.tile([128, 1], I32)
            nc.scalar.dma_start(it[:], idx_d[i, :, None])
            ghi = gpool.tile([128, L], BF16, tag="ghi")
            nc.gpsimd.indirect_dma_start(
                out=ghi[:], out_offset=None, in_=m2hi_d[:],
                in_offset=bass.IndirectOffsetOnAxis(ap=it[:, :1], axis=0))
            glo = gpool.tile([128, L], BF16, tag="glo")
            nc.gpsimd.indirect_dma_start(
                out=glo[:], out_offset=None, in_=m2lo_d[:],
                in_offset=bass.IndirectOffsetOnAxis(ap=it[:, :1], axis=0))
            orow = opool.tile([128, L * E], F32)
            gtrow = gtpool.tile([100, 4 * 128], BF16)
            nc.scalar.dma_start(gtrow[96:100, :], mrow_d[i])
            for gi in range(4):
                sl = slice(32 * gi, 32 * (gi + 1))
                gsl = slice(128 * gi, 128 * (gi + 1))
                pt = pst.tile([96, 128], BF16)
                nc.tensor.transpose(out=pt[0:32, :], in_=ghi[:, sl],
                                    identity=ident[:])
                nc.tensor.transpose(out=pt[32:64, :], in_=glo[:, sl],
                                    identity=ident[:])
                nc.tensor.transpose(out=pt[64:96, :], in_=ghi[:, sl],
                                    identity=ident[:])
                nc.vector.tensor_copy(out=gtrow[0:96, gsl], in_=pt[:])
                pos = []
                for s in range(4):
                    po = pso.tile([128, 8 * E], F32, tag="po")
                    nc.tensor.matmul(po[:], lhsT=gtrow[:, gsl],
                                     rhs=rhs_tiles[s][:],
                                     start=True, stop=True)
                    pos.append(po)
                for s in range(4):
                    win = 2048 * gi + 512 * s
                    dst = orow[:, win:win + 512]
                    if s < 2:
                        nc.vector.tensor_copy(out=dst, in_=pos[s][:])
                    else:
                        nc.scalar.copy(out=dst, in_=pos[s][:])
                nc.sync.dma_start(out_d[i][:, 2048 * gi:2048 * (gi + 1)],
                                  orow[:, 2048 * gi:2048 * (gi + 1)])
    nc.compile()
    return nc


def kernel(traj_loc, mat2, vec, traj_len, l_max, emb_sl_w, emb_su_w,
           emb_tl_w, emb_tu_w):
    import ml_dtypes
    from concourse import bass_utils

    BF = ml_dtypes.bfloat16
    traj_loc = np.asarray(traj_loc).astype(np.int64)
    mat2 = np.ascontiguousarray(np.asarray(mat2, dtype=np.float32))
    traj_len = np.asarray(traj_len).astype(np.int64)
    esl = np.asarray(emb_sl_w, dtype=np.float32)
    esu = np.asarray(emb_su_w, dtype=np.float32)
    etl = np.asarray(emb_tl_w, dtype=np.float32)
    etu = np.asarray(emb_tu_w, dtype=np.float32)

    # host prep: constants
    A = esl + etl                                            # [2, E]
    B = (esu - esl) / np.float32(SU) + (etu - etl) / np.float32(TU)
    mask = (np.arange(M)[None, :] < traj_len[:, None])       # [N, M]
    idx_full = np.where(mask, traj_loc - 1, NLOC).astype(np.int32)

    # bf16 hi/lo splits
    def split(x):
        hi = x.astype(BF)
        lo = (x - hi.astype(np.float32)).astype(BF)
        return hi, lo

    mat2x = np.concatenate([mat2, np.zeros((1, L), np.float32)], axis=0)
    m2hi, m2lo = split(mat2x)
    b1hi, b1lo = split(B[1])
    dA = A[1] - A[0]
    dAhi, dAlo = split(dA)
    a0hi, a0lo = split(A[0])

    # rhs[s] is [100, 8E]: rows 0-31 pair with GThi (x b1hi), rows 32-63
    # with GTlo (x b1hi), rows 64-95 with GThi again (x b1lo); row
    # 32*t+8*s+lp selects l' = lp within the window and scales e-block lp.
    # Rows 96-99 pair with lhsT rows [m, m, 1, 1]: m*dAhi + m*dAlo +
    # A0hi + A0lo, replicated across all 8 e-blocks.
    rhs = np.zeros((4, 100, 8 * E), BF)
    for s in range(4):
        for lp in range(8):
            rhs[s, 8 * s + lp, E * lp:E * (lp + 1)] = b1hi
            rhs[s, 32 + 8 * s + lp, E * lp:E * (lp + 1)] = b1hi
            rhs[s, 64 + 8 * s + lp, E * lp:E * (lp + 1)] = b1lo
        rhs[s, 96, :] = np.tile(dAhi, 8)
        rhs[s, 97, :] = np.tile(dAlo, 8)
        rhs[s, 98, :] = np.tile(a0hi, 8)
        rhs[s, 99, :] = np.tile(a0lo, 8)
    ident = np.eye(128, dtype=np.float32).astype(BF)

    # mrow[i] = [m, m, 1, 1] rows for lhsT rows 96-99, tiled 4x along the
    # free dim so one DMA fills all four gt windows of a row's wide tile.
    mrow_full = np.empty((N, 4, 4 * M), BF)
    mbf4 = np.tile(mask.astype(BF), (1, 4))
    mrow_full[:, 0, :] = mbf4
    mrow_full[:, 1, :] = mbf4
    mrow_full[:, 2, :] = np.ones((1, 4 * M), BF)
    mrow_full[:, 3, :] = np.ones((1, 4 * M), BF)

    if "nc" not in _CACHE:
        _CACHE["nc"] = _build()
    nc = _CACHE["nc"]

    in_maps = []
    for c in range(NCORES):
        sl = slice(ROWS * c, ROWS * (c + 1))
        in_maps.append({
            "m2hi": m2hi,
            "m2lo": m2lo,
            "idx": np.ascontiguousarray(idx_full[sl]),
            "mrow": np.ascontiguousarray(mrow_full[sl]),
            "rhs": rhs,
            "ident": ident,
        })

    trace = os.environ.get("KERNEL_TRACE", "0") == "1" and _install_profhook()
    res = bass_utils.run_bass_kernel_spmd(
        nc, in_maps, core_ids=list(range(NCORES)), trace=bool(trace))
    if trace:
        _CACHE["exec_time_ns"] = res.exec_time_ns
        _CACHE["trace_path"] = (res.instructions_and_trace or (None, None))[1]
        _CACHE["tmpdir"] = res.profile_json

    out = np.concatenate(
        [res.results[c]["out"].reshape(ROWS, M, L, E) for c in range(NCORES)],
        axis=0)
    return out

